# revision 1
# baseline (speedup 1.0000x reference)
"""Trainium2 Bass kernel for FeatureTransformerSlice (embedding lookup).

out[b, :] = bias + sum_f mask(idx[b,f]) * val[b,f] * weight[max(idx[b,f],0), :]

Strategy (8 NeuronCores, data-parallel over batch):
  - Each core owns B/8 = 2048 batch rows; the weight table [40960, 512] f32
    stays in that core's HBM and is gathered row-by-row (2KB rows) with
    indirect_dma_start.  On real TRN2 the SWDGE ucode consumes ONE offset per
    offset-AP partition and fills that dest partition's free extent
    contiguously, so each call gathers 128 random rows ([128,1] offsets into a
    whole [128, 512] tile); 512 calls/core, ~1.2us of GpSimd engine time each
    (the measured bottleneck; DMA engines sustain ~160GB/s of random 2KB rows
    underneath).
  - Per 128-row batch tile the 32 features accumulate as:
      PSUM  = ones(K=1)-matmul bias broadcast
            + PE diag(val) matmuls for PE_CHUNKS*8 features (diagonals built
              on DVE as val-broadcast * replicated-identity mask; fp32 matmul
              runs at 4 cyc/row so PE takes only half the slices)
      acc   = DVE tensor_scalar(g, val[:,f]) + add chain for the rest
      out   = PSUM + acc   (exact fp32 everywhere)
  - int64 indices are viewed as int32 pairs on the host (pure bitcast); the
    device extracts the low words, computes the >=0 mask (masked features get
    val=0) and clamps indices for the gather.

Measured: rel err ~3e-7 vs the fp32 reference, ~795us HW exec (all 8 cores).
"""

import numpy as np

P = 128
B = 16384
F = 32
V = 40960
O = 512
NCORES = 8
BC = B // NCORES          # rows per core
TILES = BC // P           # batch tiles per core
FCHUNK = 8
NCHUNKS = F // FCHUNK

# tuning knobs
PE_CHUNKS = 2             # feature chunks on PE; rest on DVE (tensor_scalar+add)
PE_DTYPE = "float32"      # "float32" (exact) or "float32r" (needs rounding pass)
G_BUFS = 28
SWDGE_QUEUES = 1       # >1 spreads indirect DMAs over SWDGE rings; measured
                       # no-op on this runtime (all traffic maps back to ring 0)


def _indirect_on_queue(nc, mybir, queue_i, **kwargs):
    """indirect_dma_start pinned to SWDGE ring `queue_i` (bass hardcodes ring 0;
    the class is restored immediately so later isinstance checks are unaffected)."""
    if queue_i == 0:
        return nc.gpsimd.indirect_dma_start(**kwargs)
    orig = mybir.InstDMACopy

    def patched(*a, **k):
        if k.get("queue") == "qPoolDynamic":
            k["queue"] = f"qPoolDynamic{queue_i}"
        return orig(*a, **k)

    mybir.InstDMACopy = patched
    try:
        return nc.gpsimd.indirect_dma_start(**kwargs)
    finally:
        mybir.InstDMACopy = orig


def build_kernel(bc=BC, tiles=None, v=V, idx_words=2, pe_chunks=PE_CHUNKS,
                 pe_dtype=PE_DTYPE, g_bufs=G_BUFS, swdge_queues=SWDGE_QUEUES):
    import concourse.bacc as bacc
    import concourse.bass as bass
    import concourse.mybir as mybir
    import concourse.tile as tile

    if tiles is None:
        tiles = bc // P
    assert bc == tiles * P

    f32 = mybir.dt.float32
    i32 = mybir.dt.int32
    mm_dt = getattr(mybir.dt, pe_dtype)

    nc = bacc.Bacc("TRN2", target_bir_lowering=False, debug=False,
                   num_swdge_queues=swdge_queues)

    idx_d = nc.dram_tensor("idx", [bc, F * idx_words], i32, kind="ExternalInput")
    val_d = nc.dram_tensor("val", [bc, F], f32, kind="ExternalInput")
    w_d = nc.dram_tensor("w", [v, O], f32, kind="ExternalInput")
    b_d = nc.dram_tensor("b", [1, O], f32, kind="ExternalInput")
    out_d = nc.dram_tensor("out", [bc, O], f32, kind="ExternalOutput")

    with tile.TileContext(nc) as tc:
        with (
            tc.tile_pool(name="io", bufs=1) as io,
            tc.tile_pool(name="gp", bufs=g_bufs) as gp,
            tc.tile_pool(name="dp", bufs=6) as dp,
            tc.tile_pool(name="ob", bufs=3) as ob,
            tc.tile_pool(name="ps", bufs=4, space="PSUM") as ps,
        ):
            # ---- one-time loads & index preprocessing ----
            idx_raw = io.tile([P, tiles, F * idx_words], i32)
            nc.sync.dma_start(
                out=idx_raw[:],
                in_=idx_d.ap().rearrange("(j p) c -> p j c", p=P),
            )
            valt = io.tile([P, tiles, F], f32)
            nc.sync.dma_start(
                out=valt[:],
                in_=val_d.ap().rearrange("(j p) f -> p j f", p=P),
            )
            bias_sb = io.tile([1, O], f32)
            nc.sync.dma_start(out=bias_sb[:], in_=b_d.ap())
            ones_sb = io.tile([1, P], f32)
            nc.vector.memset(ones_sb[:], 1.0)
            maskrep = io.tile([P, FCHUNK, P], f32)
            nc.gpsimd.memset(maskrep[:], 1.0)
            nc.gpsimd.affine_select(
                out=maskrep[:], in_=maskrep[:],
                compare_op=mybir.AluOpType.is_equal, fill=0.0, base=0,
                pattern=[[0, FCHUNK], [-1, P]], channel_multiplier=1,
            )

            nf = tiles * F
            idx32 = io.tile([P, nf], i32)
            if idx_words == 2:
                nc.vector.tensor_copy(
                    out=idx32[:].rearrange("p (j f) -> p j f", f=F).unsqueeze(3),
                    in_=idx_raw[:].rearrange("p j (f two) -> p j f two", two=2)[:, :, :, 0:1],
                )
            else:
                nc.vector.tensor_copy(out=idx32[:], in_=idx_raw[:].rearrange("p j f -> p (j f)"))

            idx_f = io.tile([P, nf], f32)
            nc.vector.tensor_copy(out=idx_f[:], in_=idx32[:])
            mask = io.tile([P, nf], f32)
            nc.vector.tensor_scalar(
                out=mask[:], in0=idx_f[:], scalar1=0.0, scalar2=None,
                op0=mybir.AluOpType.is_ge,
            )
            val_m = io.tile([P, nf], f32)
            nc.vector.tensor_tensor(
                out=val_m[:], in0=valt[:].rearrange("p j f -> p (j f)"),
                in1=mask[:], op=mybir.AluOpType.mult,
            )
            safe_f = io.tile([P, nf], f32)
            nc.vector.tensor_scalar(
                out=safe_f[:], in0=idx_f[:], scalar1=0.0, scalar2=None,
                op0=mybir.AluOpType.max,
            )
            safe_idx = io.tile([P, nf], i32)
            nc.vector.tensor_copy(out=safe_idx[:], in_=safe_f[:])

            # ---- main loop over batch tiles ----
            for t in range(tiles):
                psum = ps.tile([P, O], f32)
                # PSUM <- bias (broadcast over partitions via K=1 matmul)
                nc.tensor.matmul(
                    out=psum[:], lhsT=ones_sb[:], rhs=bias_sb[:],
                    start=True, stop=pe_chunks == 0,
                )
                acc = None
                for c in range(NCHUNKS):
                    col0 = t * F + c * FCHUNK
                    on_pe = c < pe_chunks
                    if on_pe:
                        d = dp.tile([P, FCHUNK, P], f32, tag="d")
                        vb = val_m[:, col0:col0 + FCHUNK].unsqueeze(2)
                        nc.vector.tensor_tensor(
                            out=d[:], in0=vb.to_broadcast([P, FCHUNK, P]),
                            in1=maskrep[:], op=mybir.AluOpType.mult,
                        )
                    for f in range(FCHUNK):
                        # HW SWDGE consumes ONE offset per offset-AP partition
                        # and fills that dest partition's whole free extent
                        # contiguously: one table row per partition per call,
                        # dest must be an entire [P, O] tile.
                        g = gp.tile([P, O], f32, tag="g")
                        _indirect_on_queue(
                            nc, mybir, (t * F + c * FCHUNK + f) % swdge_queues,
                            out=g[:],
                            out_offset=None,
                            in_=w_d.ap(),
                            in_offset=bass.IndirectOffsetOnAxis(
                                ap=safe_idx[:, col0 + f:col0 + f + 1], axis=0,
                            ),
                        )
                        if on_pe:
                            nc.tensor.matmul(
                                out=psum[:], lhsT=d[:, f:f + 1, :], rhs=g[:],
                                start=False,
                                stop=c == min(pe_chunks, NCHUNKS) - 1
                                and f == FCHUNK - 1,
                            )
                        else:
                            sc = dp.tile([P, O], f32, tag="s")
                            nc.vector.tensor_scalar(
                                out=sc[:], in0=g[:],
                                scalar1=val_m[:, col0 + f:col0 + f + 1],
                                scalar2=None, op0=mybir.AluOpType.mult,
                            )
                            if acc is None:
                                acc = ob.tile([P, O], f32, tag="a")
                                nc.vector.tensor_copy(out=acc[:], in_=sc[:])
                            else:
                                nc.vector.tensor_tensor(
                                    out=acc[:], in0=acc[:], in1=sc[:],
                                    op=mybir.AluOpType.add,
                                )

                out_sb = ob.tile([P, O], f32, tag="o")
                if acc is not None:
                    nc.vector.tensor_tensor(
                        out=out_sb[:], in0=psum[:], in1=acc[:],
                        op=mybir.AluOpType.add,
                    )
                else:
                    nc.vector.tensor_copy(out=out_sb[:], in_=psum[:])
                nc.sync.dma_start(
                    out=out_d.ap()[t * P:(t + 1) * P, :], in_=out_sb[:],
                )

    nc.compile()
    return nc


_nc_cache = {}


def _get_nc(idx_words):
    key = idx_words
    if key not in _nc_cache:
        _nc_cache[key] = build_kernel(idx_words=idx_words)
    return _nc_cache[key]


def _prep_in_maps(feature_indices, feature_values, weight, bias):
    fi = np.ascontiguousarray(np.asarray(feature_indices))
    fv = np.ascontiguousarray(np.asarray(feature_values), dtype=np.float32)
    w = np.ascontiguousarray(np.asarray(weight), dtype=np.float32)
    b = np.ascontiguousarray(np.asarray(bias), dtype=np.float32).reshape(1, O)

    if fi.dtype == np.int64:
        idx_words = 2
        fi32 = fi.view(np.int32).reshape(B, F * 2)
    elif fi.dtype == np.int32:
        idx_words = 1
        fi32 = fi
    else:
        fi32 = fi.astype(np.int64).view(np.int32).reshape(B, F * 2)
        idx_words = 2

    in_maps = []
    for c in range(NCORES):
        sl = slice(c * BC, (c + 1) * BC)
        in_maps.append({
            "idx": np.ascontiguousarray(fi32[sl]),
            "val": np.ascontiguousarray(fv[sl]),
            "w": w,
            "b": b,
        })
    return idx_words, in_maps


def _ensure_ntff_hook():
    """The agent image lacks antenv.axon_hooks; synthesize it (best effort) so
    a trace=True run (or a stray BASS_TRACE=1 env) never crashes on import."""
    import sys
    import types
    if "antenv.axon_hooks" in sys.modules:
        return
    try:
        from trn_agent_boot.trn_boot import _ntff_profile_via_ctypes
        hook = _ntff_profile_via_ctypes("/opt/axon/libaxon_pjrt.so")
    except Exception:
        hook = None
    try:
        mod = types.ModuleType("antenv.axon_hooks")
        mod.get_axon_ntff_profile_hook = lambda: hook
        mod.set_axon_ntff_profile_hook = lambda h: None
        sys.modules["antenv.axon_hooks"] = mod
        import antenv
        antenv.axon_hooks = mod
    except Exception:
        pass
    try:
        from concourse import bass_utils
        bass_utils.upload_artifacts = lambda tmpdir: tmpdir  # no S3 in sandbox
    except Exception:
        pass


def run_on_hw(feature_indices, feature_values, weight, bias, trace=False):
    from concourse import bass_utils
    _ensure_ntff_hook()
    idx_words, in_maps = _prep_in_maps(feature_indices, feature_values, weight, bias)
    nc = _get_nc(idx_words)
    res = bass_utils.run_bass_kernel_spmd(
        nc, in_maps, core_ids=list(range(NCORES)), trace=trace,
    )
    out = np.concatenate([r["out"] for r in res.results], axis=0)
    return out, res


def kernel(feature_indices, feature_values, weight, bias):
    out, _ = run_on_hw(feature_indices, feature_values, weight, bias, trace=False)
    return out



# revision 3
# speedup vs baseline: 1.5208x; 1.5208x over previous
"""Trainium2 Bass kernel for FeatureTransformerSlice (embedding lookup).

out[b, :] = bias + sum_f mask(idx[b,f]) * val[b,f] * weight[max(idx[b,f],0), :]

Strategy (8 NeuronCores, data-parallel over batch):
  - Each core owns B/8 = 2048 batch rows, split into NSHARDS=2 shards of 1024
    rows.  Per shard the host remaps the used vocab ids (np.unique, ~22.6K <
    int16 max) to a compact bf16 table W[uniq] that lives in that core's HBM;
    bf16 halves the random-gather HBM traffic (1KB rows) and the 2e-2 rel-err
    budget dwarfs the 2^-9 rounding.
  - Gathers use the SWDGE dma_gather instruction: ONE GpSimd call fetches all
    4096 rows of a 128-row batch tile (32 features each) into a [128, 32, 512]
    bf16 SBUF tile (flat slot i = j*128+p lands at [p, j, :]).  16 calls/core
    replace the baseline's 512 indirect_dma_start calls, amortizing the ~1us
    SWDGE fixed overhead that serialized the whole kernel (GpSimd was 80% busy,
    DMA engines 50% idle waiting on descriptors).
  - Per tile the 32 features reduce on the PE as bf16 diag(val) matmuls
    accumulated in fp32 PSUM (bias enters via a K=1 ones x bias matmul);
    diagonals are built on DVE as val-broadcast * replicated-identity.  A few
    features can be peeled onto DVE (tensor_scalar + add) via DVE_FEATS to
    balance engines.  ACT copies PSUM to SBUF for the output DMA.
  - Indices arrive pre-wrapped from the host in the SWDGE layout (16-partition
    wrap replicated 8x across the 128 partitions); values arrive bf16 in
    feature-major [128, tiles*32] layout; masked (negative) features get val=0.
"""

import numpy as np
import ml_dtypes

bf16 = ml_dtypes.bfloat16

P = 128
B = 16384
F = 32
V = 40960
O = 512
NCORES = 8
BC = B // NCORES          # rows per core
TILES = BC // P           # batch tiles per core (16)
NI = P * F                # gathered rows per tile per call (4096)
S = NI // 16              # idx columns per tile in the 16-partition wrap (256)

# tuning knobs
NSHARDS = 2               # vocab-remap shards per core (2 -> ~22.6K uniq ids)
U_PAD = 23552             # padded compact-table rows (fits int16, > max uniq)
NSHARDS_FB = 4            # fallback if a shard overflows U_PAD
U_PAD_FB = 16384
DVE_FEATS = 0             # features per tile computed on DVE instead of PE
G_BUFS = 3                # gather-tile double/triple buffering
IDX_PER_CALL = 1024       # rows per dma_gather call (divisor of NI, mult of 128;
                          # <= 1024 so one call's descriptors fit the 16KB
                          # SWDGE ring carveout)


def build_kernel(nshards=NSHARDS, u_pad=U_PAD, dve_feats=DVE_FEATS,
                 g_bufs=G_BUFS, idx_per_call=IDX_PER_CALL):
    import concourse.bacc as bacc
    import concourse.bass as bass
    import concourse.mybir as mybir
    import concourse.tile as tile

    f32 = mybir.dt.float32
    bf = mybir.dt.bfloat16
    i16 = mybir.dt.int16

    tiles_per_shard = TILES // nshards
    calls_per_tile = NI // idx_per_call
    j_per_call = idx_per_call // P

    nc = bacc.Bacc("TRN2", target_bir_lowering=False, debug=False)

    w_ds = [nc.dram_tensor(f"w{h}", [u_pad, O], bf, kind="ExternalInput")
            for h in range(nshards)]
    ix_d = nc.dram_tensor("ix", [P, TILES * S], i16, kind="ExternalInput")
    vb_d = nc.dram_tensor("vb", [P, TILES * F], bf, kind="ExternalInput")
    b_d = nc.dram_tensor("b", [1, O], bf, kind="ExternalInput")
    out_d = nc.dram_tensor("out", [BC, O], f32, kind="ExternalOutput")

    with tile.TileContext(nc) as tc:
        with (
            tc.tile_pool(name="io", bufs=1) as io,
            tc.tile_pool(name="gp", bufs=g_bufs) as gp,
            tc.tile_pool(name="dp", bufs=3) as dp,
            tc.tile_pool(name="ob", bufs=3) as ob,
            tc.tile_pool(name="ps", bufs=4, space="PSUM") as ps,
        ):
            # ---- one-time loads ----
            ix_sb = io.tile([P, TILES * S], i16)
            nc.sync.dma_start(out=ix_sb[:], in_=ix_d.ap())
            vb_sb = io.tile([P, TILES * F], bf)
            nc.sync.dma_start(out=vb_sb[:], in_=vb_d.ap())
            bias_sb = io.tile([1, O], bf)
            nc.sync.dma_start(out=bias_sb[:], in_=b_d.ap())
            ones_sb = io.tile([1, P], bf)
            nc.vector.memset(ones_sb[:], 1.0)
            pe_feats = F - dve_feats
            maskrep = io.tile([P, pe_feats, P], bf)
            nc.gpsimd.memset(maskrep[:], 1.0)
            nc.gpsimd.affine_select(
                out=maskrep[:], in_=maskrep[:],
                compare_op=mybir.AluOpType.is_equal, fill=0.0, base=0,
                pattern=[[0, pe_feats], [-1, P]], channel_multiplier=1,
            )

            # ---- main loop over batch tiles ----
            for t in range(TILES):
                w_d = w_ds[t // tiles_per_shard]
                G = gp.tile([P, F, O], bf, tag="g")
                for cc in range(calls_per_tile):
                    nc.gpsimd.dma_gather(
                        out_ap=G[:, cc * j_per_call:(cc + 1) * j_per_call, :],
                        in_ap=w_d.ap(),
                        idxs_ap=ix_sb[:, t * S + cc * (S // calls_per_tile):
                                      t * S + (cc + 1) * (S // calls_per_tile)],
                        num_idxs=idx_per_call,
                        num_idxs_reg=idx_per_call,
                        elem_size=O,
                    )

                d = dp.tile([P, pe_feats, P], bf, tag="d")
                vb_pe = vb_sb[:, t * F:t * F + pe_feats].unsqueeze(2)
                nc.vector.tensor_tensor(
                    out=d[:], in0=vb_pe.to_broadcast([P, pe_feats, P]),
                    in1=maskrep[:], op=mybir.AluOpType.mult,
                )

                psum = ps.tile([P, O], f32)
                nc.tensor.matmul(
                    out=psum[:], lhsT=ones_sb[:], rhs=bias_sb[:],
                    start=True, stop=False,
                )
                for j in range(pe_feats):
                    nc.tensor.matmul(
                        out=psum[:], lhsT=d[:, j, :], rhs=G[:, j, :],
                        start=False, stop=j == pe_feats - 1,
                    )

                acc = None
                for j in range(pe_feats, F):
                    sc = dp.tile([P, O], f32, tag="s")
                    nc.vector.tensor_scalar(
                        out=sc[:], in0=G[:, j, :],
                        scalar1=vb_sb[:, t * F + j:t * F + j + 1],
                        scalar2=None, op0=mybir.AluOpType.mult,
                    )
                    if acc is None:
                        acc = ob.tile([P, O], f32, tag="a")
                        nc.vector.tensor_copy(out=acc[:], in_=sc[:])
                    else:
                        nc.vector.tensor_tensor(
                            out=acc[:], in0=acc[:], in1=sc[:],
                            op=mybir.AluOpType.add,
                        )

                out_sb = ob.tile([P, O], f32, tag="o")
                if acc is None:
                    nc.scalar.activation(
                        out=out_sb[:], in_=psum[:],
                        func=mybir.ActivationFunctionType.Copy,
                    )
                else:
                    nc.vector.tensor_tensor(
                        out=out_sb[:], in0=psum[:], in1=acc[:],
                        op=mybir.AluOpType.add,
                    )
                nc.sync.dma_start(
                    out=out_d.ap()[t * P:(t + 1) * P, :], in_=out_sb[:],
                )

    nc.compile()
    return nc


_nc_cache = {}


def _get_nc(nshards, u_pad):
    key = (nshards, u_pad)
    if key not in _nc_cache:
        _nc_cache[key] = build_kernel(nshards=nshards, u_pad=u_pad)
    return _nc_cache[key]


def _pack_shard(idx_eff, val_eff, w_bf, u_pad):
    """Remap one shard's indices to a compact table.

    Returns (padded bf16 table [u_pad, O], wrapped int16 idx [tiles, P, S],
    bf16 val [P, tiles*F]) or None if the shard's vocab overflows u_pad."""
    rows = idx_eff.shape[0]
    tiles = rows // P
    uniq, inv = np.unique(idx_eff, return_inverse=True)
    if len(uniq) > min(u_pad, 32768):
        return None
    table = np.zeros((u_pad, O), bf16)
    table[:len(uniq)] = w_bf[uniq]
    idx16 = inv.reshape(rows, F).astype(np.int16)
    a = idx16.reshape(tiles, P, F).transpose(0, 2, 1)   # [t, j, p]: flat i=j*128+p
    a = a.reshape(tiles, NI).reshape(tiles, S, 16)      # [t, s, p16]: i=s*16+p16
    a = a.transpose(0, 2, 1)                            # [t, p16, s]
    ix = np.broadcast_to(a[:, None], (tiles, 8, 16, S)).reshape(tiles, P, S)
    v = val_eff.reshape(tiles, P, F).transpose(1, 0, 2).reshape(P, tiles * F)
    return table, np.ascontiguousarray(ix), v.astype(bf16)


def _prep_in_maps(feature_indices, feature_values, weight, bias):
    fi = np.asarray(feature_indices)
    fv = np.asarray(feature_values, dtype=np.float32)
    w_bf = np.asarray(weight, dtype=np.float32).astype(bf16)
    b_bf = np.asarray(bias, dtype=np.float32).astype(bf16).reshape(1, O)

    mask = fi >= 0
    val_eff = np.where(mask, fv, np.float32(0.0))
    idx_eff = np.where(mask, fi, 0).astype(np.int64)

    for nshards, u_pad in [(NSHARDS, U_PAD), (NSHARDS_FB, U_PAD_FB)]:
        ss = BC // nshards
        in_maps = []
        ok = True
        for c in range(NCORES):
            tables, ixs, vbs = [], [], []
            for h in range(nshards):
                lo = c * BC + h * ss
                packed = _pack_shard(idx_eff[lo:lo + ss], val_eff[lo:lo + ss],
                                     w_bf, u_pad)
                if packed is None:
                    ok = False
                    break
                tables.append(packed[0])
                ixs.append(packed[1])
                vbs.append(packed[2])
            if not ok:
                break
            m = {f"w{h}": tables[h] for h in range(nshards)}
            m["ix"] = np.concatenate(ixs, 0).transpose(1, 0, 2).reshape(
                P, TILES * S).copy()
            m["vb"] = np.concatenate(vbs, 1)
            m["b"] = b_bf
            in_maps.append(m)
        if ok:
            return nshards, u_pad, in_maps
    raise RuntimeError("vocab shard overflowed even the fallback split")


def _ensure_ntff_hook():
    """The agent image lacks antenv.axon_hooks; synthesize it (best effort) so
    a trace=True run (or a stray BASS_TRACE=1 env) never crashes on import."""
    import sys
    import types
    if "antenv.axon_hooks" in sys.modules:
        return
    try:
        from trn_agent_boot.trn_boot import _ntff_profile_via_ctypes
        hook = _ntff_profile_via_ctypes("/opt/axon/libaxon_pjrt.so")
    except Exception:
        hook = None
    try:
        mod = types.ModuleType("antenv.axon_hooks")
        mod.get_axon_ntff_profile_hook = lambda: hook
        mod.set_axon_ntff_profile_hook = lambda h: None
        sys.modules["antenv.axon_hooks"] = mod
        import antenv
        antenv.axon_hooks = mod
    except Exception:
        pass
    try:
        from concourse import bass_utils
        bass_utils.upload_artifacts = lambda tmpdir: tmpdir  # no S3 in sandbox
    except Exception:
        pass


def run_on_hw(feature_indices, feature_values, weight, bias, trace=False):
    from concourse import bass_utils
    _ensure_ntff_hook()
    nshards, u_pad, in_maps = _prep_in_maps(
        feature_indices, feature_values, weight, bias)
    nc = _get_nc(nshards, u_pad)
    res = bass_utils.run_bass_kernel_spmd(
        nc, in_maps, core_ids=list(range(NCORES)), trace=trace,
    )
    out = np.concatenate([r["out"] for r in res.results], axis=0)
    return out, res


def kernel(feature_indices, feature_values, weight, bias):
    out, _ = run_on_hw(feature_indices, feature_values, weight, bias,
                       trace=False)
    return out


# revision 9
# speedup vs baseline: 2.9938x; 1.9685x over previous
"""Trainium2 Bass kernel for FeatureTransformerSlice (embedding lookup).

out[b, :] = bias + sum_f mask(idx[b,f]) * val[b,f] * weight[max(idx[b,f],0), :]

Strategy (8 NeuronCores, data-parallel over batch):
  - Each core owns B/8 = 2048 batch rows, split into NSHARDS=2 shards of 1024
    rows.  Per shard the host remaps the used vocab ids (np.unique, ~22.6K <
    int16 max) to a compact bf16 table W[uniq] that lives in that core's HBM;
    bf16 halves the random-gather HBM traffic (1KB rows) and the 2e-2 rel-err
    budget dwarfs the 2^-9 rounding.
  - Gathers use the SWDGE dma_gather instruction: ONE GpSimd call fetches all
    4096 rows of a 128-row batch tile (32 features each) into a [128, 32, 512]
    bf16 SBUF tile (flat slot i = j*128+p lands at [p, j, :]).  16 calls/core
    replace the baseline's 512 indirect_dma_start calls, amortizing the ~1us
    SWDGE fixed overhead that serialized the whole kernel (GpSimd was 80% busy,
    DMA engines 50% idle waiting on descriptors).
  - Per tile the 32 features reduce on the PE as bf16 diag(val) matmuls
    accumulated in fp32 PSUM (bias enters via a K=1 ones x bias matmul);
    diagonals are built on DVE as val-broadcast * replicated-identity.  A few
    features can be peeled onto DVE (tensor_scalar + add) via DVE_FEATS to
    balance engines.  ACT copies PSUM to SBUF for the output DMA.
  - Indices arrive pre-wrapped from the host in the SWDGE layout (16-partition
    wrap replicated 8x across the 128 partitions); values arrive bf16 in
    feature-major [128, tiles*32] layout; masked (negative) features get val=0.
"""

import numpy as np
import ml_dtypes

bf16 = ml_dtypes.bfloat16

P = 128
B = 16384
F = 32
V = 40960
O = 512
NCORES = 8
BC = B // NCORES          # rows per core
TILES = BC // P           # batch tiles per core (16)
NI = P * F                # gathered rows per tile per call (4096)
S = NI // 16              # idx columns per tile in the 16-partition wrap (256)

# tuning knobs
NSHARDS = 2               # vocab-remap shards per core (2 -> ~22.6K uniq ids)
U_PAD = 23552             # padded compact-table rows (fits int16, > max uniq)
NSHARDS_FB = 4            # fallback if a shard overflows U_PAD
U_PAD_FB = 16384
DVE_FEATS = 8             # features per tile computed on DVE instead of PE
G_BUFS = 3                # gather-tile double/triple buffering
IDX_PER_CALL = 1024       # rows per dma_gather call (divisor of NI, mult of 128;
                          # <= 1024 so one call's descriptors fit the 16KB
                          # SWDGE ring carveout)
SWDGE_QUEUES = 4          # spread gather calls over SWDGE rings


def build_kernel(nshards=NSHARDS, u_pad=U_PAD, dve_feats=DVE_FEATS,
                 g_bufs=G_BUFS, idx_per_call=IDX_PER_CALL,
                 swdge_queues=SWDGE_QUEUES):
    import concourse.bacc as bacc
    import concourse.bass as bass
    import concourse.mybir as mybir
    import concourse.tile as tile

    f32 = mybir.dt.float32
    bf = mybir.dt.bfloat16
    i16 = mybir.dt.int16

    tiles_per_shard = TILES // nshards
    calls_per_tile = NI // idx_per_call
    j_per_call = idx_per_call // P

    nc = bacc.Bacc("TRN2", target_bir_lowering=False, debug=False,
                   num_swdge_queues=swdge_queues)

    w_ds = [nc.dram_tensor(f"w{h}", [u_pad, O], bf, kind="ExternalInput")
            for h in range(nshards)]
    ix_d = nc.dram_tensor("ix", [P, TILES * S], i16, kind="ExternalInput")
    vb_d = nc.dram_tensor("vb", [P, TILES * F], bf, kind="ExternalInput")
    b_d = nc.dram_tensor("b", [1, O], bf, kind="ExternalInput")
    out_d = nc.dram_tensor("out", [BC, O], f32, kind="ExternalOutput")

    with tile.TileContext(nc) as tc:
        with (
            tc.tile_pool(name="io", bufs=1) as io,
            tc.tile_pool(name="gp", bufs=g_bufs) as gp,
            tc.tile_pool(name="dp", bufs=3) as dp,
            tc.tile_pool(name="ob", bufs=3) as ob,
            tc.tile_pool(name="ps", bufs=4, space="PSUM") as ps,
        ):
            # ---- one-time loads ----
            ix_sb = io.tile([P, TILES * S], i16)
            nc.sync.dma_start(out=ix_sb[:], in_=ix_d.ap())
            vb_sb = io.tile([P, TILES * F], bf)
            nc.sync.dma_start(out=vb_sb[:], in_=vb_d.ap())
            bias_sb = io.tile([1, O], bf)
            nc.sync.dma_start(out=bias_sb[:], in_=b_d.ap())
            if dve_feats:
                vf_sb = io.tile([P, TILES * F], f32)
                nc.vector.tensor_copy(out=vf_sb[:], in_=vb_sb[:])
            ones_sb = io.tile([1, P], bf)
            nc.vector.memset(ones_sb[:], 1.0)
            pe_feats = F - dve_feats
            maskrep = io.tile([P, pe_feats, P], bf)
            nc.gpsimd.memset(maskrep[:], 1.0)
            nc.gpsimd.affine_select(
                out=maskrep[:], in_=maskrep[:],
                compare_op=mybir.AluOpType.is_equal, fill=0.0, base=0,
                pattern=[[0, pe_feats], [-1, P]], channel_multiplier=1,
            )

            # ---- main loop over batch tiles ----
            for t in range(TILES):
                w_d = w_ds[t // tiles_per_shard]
                G = gp.tile([P, F, O], bf, tag="g")
                for cc in range(calls_per_tile):
                    nc.gpsimd.dma_gather(
                        out_ap=G[:, cc * j_per_call:(cc + 1) * j_per_call, :],
                        in_ap=w_d.ap(),
                        idxs_ap=ix_sb[:, t * S + cc * (S // calls_per_tile):
                                      t * S + (cc + 1) * (S // calls_per_tile)],
                        num_idxs=idx_per_call,
                        num_idxs_reg=idx_per_call,
                        elem_size=O,
                        queue_num=(t * calls_per_tile + cc) % swdge_queues,
                    )

                d = dp.tile([P, pe_feats, P], bf, tag="d")
                vb_pe = vb_sb[:, t * F:t * F + pe_feats].unsqueeze(2)
                nc.vector.tensor_tensor(
                    out=d[:], in0=vb_pe.to_broadcast([P, pe_feats, P]),
                    in1=maskrep[:], op=mybir.AluOpType.mult,
                )

                psum = ps.tile([P, O], f32)
                nc.tensor.matmul(
                    out=psum[:], lhsT=ones_sb[:], rhs=bias_sb[:],
                    start=True, stop=False,
                )
                for j in range(pe_feats):
                    nc.tensor.matmul(
                        out=psum[:], lhsT=d[:, j, :], rhs=G[:, j, :],
                        start=False, stop=j == pe_feats - 1,
                    )

                acc = None
                for j in range(pe_feats, F):
                    sc = dp.tile([P, O], f32, tag="s")
                    nc.vector.tensor_scalar(
                        out=sc[:], in0=G[:, j, :],
                        scalar1=vf_sb[:, t * F + j:t * F + j + 1],
                        scalar2=None, op0=mybir.AluOpType.mult,
                    )
                    if acc is None:
                        acc = ob.tile([P, O], f32, tag="a")
                        nc.vector.tensor_copy(out=acc[:], in_=sc[:])
                    else:
                        nc.vector.tensor_tensor(
                            out=acc[:], in0=acc[:], in1=sc[:],
                            op=mybir.AluOpType.add,
                        )

                out_sb = ob.tile([P, O], f32, tag="o")
                if acc is None:
                    nc.scalar.activation(
                        out=out_sb[:], in_=psum[:],
                        func=mybir.ActivationFunctionType.Copy,
                    )
                else:
                    nc.vector.tensor_tensor(
                        out=out_sb[:], in0=psum[:], in1=acc[:],
                        op=mybir.AluOpType.add,
                    )
                nc.sync.dma_start(
                    out=out_d.ap()[t * P:(t + 1) * P, :], in_=out_sb[:],
                )

    nc.compile()
    return nc


_nc_cache = {}


def _get_nc(nshards, u_pad):
    key = (nshards, u_pad)
    if key not in _nc_cache:
        _nc_cache[key] = build_kernel(nshards=nshards, u_pad=u_pad)
    return _nc_cache[key]


def _pack_shard(idx_eff, val_eff, w_bf, u_pad):
    """Remap one shard's indices to a compact table.

    Returns (padded bf16 table [u_pad, O], wrapped int16 idx [tiles, P, S],
    bf16 val [P, tiles*F]) or None if the shard's vocab overflows u_pad."""
    rows = idx_eff.shape[0]
    tiles = rows // P
    uniq, inv = np.unique(idx_eff, return_inverse=True)
    if len(uniq) > min(u_pad, 32768):
        return None
    table = np.zeros((u_pad, O), bf16)
    table[:len(uniq)] = w_bf[uniq]
    idx16 = inv.reshape(rows, F).astype(np.int16)
    a = idx16.reshape(tiles, P, F).transpose(0, 2, 1)   # [t, j, p]: flat i=j*128+p
    a = a.reshape(tiles, NI).reshape(tiles, S, 16)      # [t, s, p16]: i=s*16+p16
    a = a.transpose(0, 2, 1)                            # [t, p16, s]
    ix = np.broadcast_to(a[:, None], (tiles, 8, 16, S)).reshape(tiles, P, S)
    v = val_eff.reshape(tiles, P, F).transpose(1, 0, 2).reshape(P, tiles * F)
    return table, np.ascontiguousarray(ix), v.astype(bf16)


def _prep_in_maps(feature_indices, feature_values, weight, bias):
    fi = np.asarray(feature_indices)
    fv = np.asarray(feature_values, dtype=np.float32)
    w_bf = np.asarray(weight, dtype=np.float32).astype(bf16)
    b_bf = np.asarray(bias, dtype=np.float32).astype(bf16).reshape(1, O)

    mask = fi >= 0
    val_eff = np.where(mask, fv, np.float32(0.0))
    idx_eff = np.where(mask, fi, 0).astype(np.int64)

    for nshards, u_pad in [(NSHARDS, U_PAD), (NSHARDS_FB, U_PAD_FB)]:
        ss = BC // nshards
        in_maps = []
        ok = True
        for c in range(NCORES):
            tables, ixs, vbs = [], [], []
            for h in range(nshards):
                lo = c * BC + h * ss
                packed = _pack_shard(idx_eff[lo:lo + ss], val_eff[lo:lo + ss],
                                     w_bf, u_pad)
                if packed is None:
                    ok = False
                    break
                tables.append(packed[0])
                ixs.append(packed[1])
                vbs.append(packed[2])
            if not ok:
                break
            m = {f"w{h}": tables[h] for h in range(nshards)}
            m["ix"] = np.concatenate(ixs, 0).transpose(1, 0, 2).reshape(
                P, TILES * S).copy()
            m["vb"] = np.concatenate(vbs, 1)
            m["b"] = b_bf
            in_maps.append(m)
        if ok:
            return nshards, u_pad, in_maps
    raise RuntimeError("vocab shard overflowed even the fallback split")


def _ensure_ntff_hook():
    """The agent image lacks antenv.axon_hooks; synthesize it (best effort) so
    a trace=True run (or a stray BASS_TRACE=1 env) never crashes on import."""
    import sys
    import types
    if "antenv.axon_hooks" in sys.modules:
        return
    try:
        from trn_agent_boot.trn_boot import _ntff_profile_via_ctypes
        hook = _ntff_profile_via_ctypes("/opt/axon/libaxon_pjrt.so")
    except Exception:
        hook = None
    try:
        mod = types.ModuleType("antenv.axon_hooks")
        mod.get_axon_ntff_profile_hook = lambda: hook
        mod.set_axon_ntff_profile_hook = lambda h: None
        sys.modules["antenv.axon_hooks"] = mod
        import antenv
        antenv.axon_hooks = mod
    except Exception:
        pass
    try:
        from concourse import bass_utils
        bass_utils.upload_artifacts = lambda tmpdir: tmpdir  # no S3 in sandbox
    except Exception:
        pass


def run_on_hw(feature_indices, feature_values, weight, bias, trace=False):
    from concourse import bass_utils
    _ensure_ntff_hook()
    nshards, u_pad, in_maps = _prep_in_maps(
        feature_indices, feature_values, weight, bias)
    nc = _get_nc(nshards, u_pad)
    res = bass_utils.run_bass_kernel_spmd(
        nc, in_maps, core_ids=list(range(NCORES)), trace=trace,
    )
    out = np.concatenate([r["out"] for r in res.results], axis=0)
    return out, res


def kernel(feature_indices, feature_values, weight, bias):
    out, _ = run_on_hw(feature_indices, feature_values, weight, bias,
                       trace=False)
    return out


# revision 13
# speedup vs baseline: 3.0129x; 1.0064x over previous
"""Trainium2 Bass kernel for FeatureTransformerSlice (embedding lookup).

out[b, :] = bias + sum_f mask(idx[b,f]) * val[b,f] * weight[max(idx[b,f],0), :]

Strategy (8 NeuronCores, data-parallel over batch):
  - Each core owns B/8 = 2048 batch rows, split into NSHARDS=2 shards of 1024
    rows.  Per shard the host remaps the used vocab ids (np.unique, ~22.6K <
    int16 max) to a compact bf16 table W[uniq] that lives in that core's HBM;
    bf16 halves the random-gather HBM traffic (1KB rows) and the 2e-2 rel-err
    budget dwarfs the 2^-9 rounding.
  - Gathers use the SWDGE dma_gather instruction: ONE GpSimd call fetches all
    4096 rows of a 128-row batch tile (32 features each) into a [128, 32, 512]
    bf16 SBUF tile (flat slot i = j*128+p lands at [p, j, :]).  16 calls/core
    replace the baseline's 512 indirect_dma_start calls, amortizing the ~1us
    SWDGE fixed overhead that serialized the whole kernel (GpSimd was 80% busy,
    DMA engines 50% idle waiting on descriptors).
  - Per tile the 32 features reduce on the PE as bf16 diag(val) matmuls
    accumulated in fp32 PSUM (bias enters via a K=1 ones x bias matmul);
    diagonals are built on DVE as val-broadcast * replicated-identity.  A few
    features can be peeled onto DVE (tensor_scalar + add) via DVE_FEATS to
    balance engines.  ACT copies PSUM to SBUF for the output DMA.
  - Indices arrive pre-wrapped from the host in the SWDGE layout (16-partition
    wrap replicated 8x across the 128 partitions); values arrive bf16 in
    feature-major [128, tiles*32] layout; masked (negative) features get val=0.
"""

import numpy as np
import ml_dtypes

bf16 = ml_dtypes.bfloat16

P = 128
B = 16384
F = 32
V = 40960
O = 512
NCORES = 8
BC = B // NCORES          # rows per core
TILES = BC // P           # batch tiles per core (16)
NI = P * F                # gathered rows per tile per call (4096)
S = NI // 16              # idx columns per tile in the 16-partition wrap (256)

# tuning knobs
NSHARDS = 2               # vocab-remap shards per core (2 -> ~22.6K uniq ids)
U_PAD = 23552             # padded compact-table rows (fits int16, > max uniq)
NSHARDS_FB = 4            # fallback if a shard overflows U_PAD
U_PAD_FB = 16384
DVE_FEATS = 16            # features per tile computed on DVE instead of PE
                          # (power of two; reduced via a binary tree of wide
                          # bf16 tensor_tensor ops)
G_BUFS = 3                # gather-tile double/triple buffering
IDX_PER_CALL = 1024       # rows per dma_gather call (divisor of NI, mult of 128;
                          # <= 1024 so one call's descriptors fit the 16KB
                          # SWDGE ring carveout)
SWDGE_QUEUES = 4          # spread gather calls over SWDGE rings


def build_kernel(nshards=NSHARDS, u_pad=U_PAD, dve_feats=DVE_FEATS,
                 g_bufs=G_BUFS, idx_per_call=IDX_PER_CALL,
                 swdge_queues=SWDGE_QUEUES):
    import concourse.bacc as bacc
    import concourse.bass as bass
    import concourse.mybir as mybir
    import concourse.tile as tile

    f32 = mybir.dt.float32
    bf = mybir.dt.bfloat16
    i16 = mybir.dt.int16

    tiles_per_shard = TILES // nshards
    calls_per_tile = NI // idx_per_call
    j_per_call = idx_per_call // P

    nc = bacc.Bacc("TRN2", target_bir_lowering=False, debug=False,
                   num_swdge_queues=swdge_queues)

    w_ds = [nc.dram_tensor(f"w{h}", [u_pad, O], bf, kind="ExternalInput")
            for h in range(nshards)]
    ix_d = nc.dram_tensor("ix", [P, TILES * S], i16, kind="ExternalInput")
    vb_d = nc.dram_tensor("vb", [P, TILES * F], bf, kind="ExternalInput")
    b_d = nc.dram_tensor("b", [1, O], bf, kind="ExternalInput")
    out_d = nc.dram_tensor("out", [BC, O], f32, kind="ExternalOutput")

    with tile.TileContext(nc) as tc:
        with (
            tc.tile_pool(name="io", bufs=1) as io,
            tc.tile_pool(name="gp", bufs=g_bufs) as gp,
            tc.tile_pool(name="dp", bufs=2) as dp,
            tc.tile_pool(name="ob", bufs=3) as ob,
            tc.tile_pool(name="ps", bufs=4, space="PSUM") as ps,
        ):
            # ---- one-time loads ----
            ix_sb = io.tile([P, TILES * S], i16)
            nc.sync.dma_start(out=ix_sb[:], in_=ix_d.ap())
            vb_sb = io.tile([P, TILES * F], bf)
            nc.sync.dma_start(out=vb_sb[:], in_=vb_d.ap())
            bias_sb = io.tile([1, O], bf)
            nc.sync.dma_start(out=bias_sb[:], in_=b_d.ap())
            assert dve_feats == 0 or (dve_feats >= 2
                                      and dve_feats & (dve_feats - 1) == 0)
            ones_sb = io.tile([1, P], bf)
            nc.vector.memset(ones_sb[:], 1.0)
            pe_feats = F - dve_feats
            maskrep = io.tile([P, pe_feats, P], bf)
            nc.gpsimd.memset(maskrep[:], 1.0)
            nc.gpsimd.affine_select(
                out=maskrep[:], in_=maskrep[:],
                compare_op=mybir.AluOpType.is_equal, fill=0.0, base=0,
                pattern=[[0, pe_feats], [-1, P]], channel_multiplier=1,
            )

            # ---- main loop over batch tiles ----
            for t in range(TILES):
                w_d = w_ds[t // tiles_per_shard]
                G = gp.tile([P, F, O], bf, tag="g")
                for cc in range(calls_per_tile):
                    nc.gpsimd.dma_gather(
                        out_ap=G[:, cc * j_per_call:(cc + 1) * j_per_call, :],
                        in_ap=w_d.ap(),
                        idxs_ap=ix_sb[:, t * S + cc * (S // calls_per_tile):
                                      t * S + (cc + 1) * (S // calls_per_tile)],
                        num_idxs=idx_per_call,
                        num_idxs_reg=idx_per_call,
                        elem_size=O,
                        queue_num=(t * calls_per_tile + cc) % swdge_queues,
                    )

                d = dp.tile([P, pe_feats, P], bf, tag="d")
                vb_pe = vb_sb[:, t * F:t * F + pe_feats].unsqueeze(2)
                nc.vector.tensor_tensor(
                    out=d[:], in0=vb_pe.to_broadcast([P, pe_feats, P]),
                    in1=maskrep[:], op=mybir.AluOpType.mult,
                )

                psum = ps.tile([P, O], f32)
                nc.tensor.matmul(
                    out=psum[:], lhsT=ones_sb[:], rhs=bias_sb[:],
                    start=True, stop=False,
                )
                for j in range(pe_feats):
                    nc.tensor.matmul(
                        out=psum[:], lhsT=d[:, j, :], rhs=G[:, j, :],
                        start=False, stop=j == pe_feats - 1,
                    )

                acc = None
                if dve_feats:
                    sc = dp.tile([P, dve_feats, O], bf, tag="s")
                    vb_dv = vb_sb[:, t * F + pe_feats:t * F + F].unsqueeze(2)
                    nc.vector.tensor_tensor(
                        out=sc[:], in0=vb_dv.to_broadcast([P, dve_feats, O]),
                        in1=G[:, pe_feats:F, :], op=mybir.AluOpType.mult,
                    )
                    cur, width, lvl = sc, dve_feats, 0
                    while width > 2:
                        half = width // 2
                        nxt = dp.tile([P, half, O], bf, tag=f"t{lvl}")
                        nc.vector.tensor_tensor(
                            out=nxt[:], in0=cur[:, :half, :],
                            in1=cur[:, half:width, :], op=mybir.AluOpType.add,
                        )
                        cur, width, lvl = nxt, half, lvl + 1
                    acc = ob.tile([P, O], f32, tag="a")
                    nc.vector.tensor_tensor(
                        out=acc[:], in0=cur[:, 0, :], in1=cur[:, 1, :],
                        op=mybir.AluOpType.add,
                    )

                out_sb = ob.tile([P, O], f32, tag="o")
                if acc is None:
                    nc.scalar.activation(
                        out=out_sb[:], in_=psum[:],
                        func=mybir.ActivationFunctionType.Copy,
                    )
                else:
                    nc.vector.tensor_tensor(
                        out=out_sb[:], in0=psum[:], in1=acc[:],
                        op=mybir.AluOpType.add,
                    )
                nc.sync.dma_start(
                    out=out_d.ap()[t * P:(t + 1) * P, :], in_=out_sb[:],
                )

    nc.compile()
    return nc


_nc_cache = {}


def _get_nc(nshards, u_pad):
    key = (nshards, u_pad)
    if key not in _nc_cache:
        _nc_cache[key] = build_kernel(nshards=nshards, u_pad=u_pad)
    return _nc_cache[key]


def _pack_shard(idx_eff, val_eff, w_bf, u_pad):
    """Remap one shard's indices to a compact table.

    Returns (padded bf16 table [u_pad, O], wrapped int16 idx [tiles, P, S],
    bf16 val [P, tiles*F]) or None if the shard's vocab overflows u_pad."""
    rows = idx_eff.shape[0]
    tiles = rows // P
    uniq, inv = np.unique(idx_eff, return_inverse=True)
    if len(uniq) > min(u_pad, 32768):
        return None
    table = np.zeros((u_pad, O), bf16)
    table[:len(uniq)] = w_bf[uniq]
    idx16 = inv.reshape(rows, F).astype(np.int16)
    a = idx16.reshape(tiles, P, F).transpose(0, 2, 1)   # [t, j, p]: flat i=j*128+p
    a = a.reshape(tiles, NI).reshape(tiles, S, 16)      # [t, s, p16]: i=s*16+p16
    a = a.transpose(0, 2, 1)                            # [t, p16, s]
    ix = np.broadcast_to(a[:, None], (tiles, 8, 16, S)).reshape(tiles, P, S)
    v = val_eff.reshape(tiles, P, F).transpose(1, 0, 2).reshape(P, tiles * F)
    return table, np.ascontiguousarray(ix), v.astype(bf16)


def _prep_in_maps(feature_indices, feature_values, weight, bias):
    fi = np.asarray(feature_indices)
    fv = np.asarray(feature_values, dtype=np.float32)
    w_bf = np.asarray(weight, dtype=np.float32).astype(bf16)
    b_bf = np.asarray(bias, dtype=np.float32).astype(bf16).reshape(1, O)

    mask = fi >= 0
    val_eff = np.where(mask, fv, np.float32(0.0))
    idx_eff = np.where(mask, fi, 0).astype(np.int64)

    for nshards, u_pad in [(NSHARDS, U_PAD), (NSHARDS_FB, U_PAD_FB)]:
        ss = BC // nshards
        in_maps = []
        ok = True
        for c in range(NCORES):
            tables, ixs, vbs = [], [], []
            for h in range(nshards):
                lo = c * BC + h * ss
                packed = _pack_shard(idx_eff[lo:lo + ss], val_eff[lo:lo + ss],
                                     w_bf, u_pad)
                if packed is None:
                    ok = False
                    break
                tables.append(packed[0])
                ixs.append(packed[1])
                vbs.append(packed[2])
            if not ok:
                break
            m = {f"w{h}": tables[h] for h in range(nshards)}
            m["ix"] = np.concatenate(ixs, 0).transpose(1, 0, 2).reshape(
                P, TILES * S).copy()
            m["vb"] = np.concatenate(vbs, 1)
            m["b"] = b_bf
            in_maps.append(m)
        if ok:
            return nshards, u_pad, in_maps
    raise RuntimeError("vocab shard overflowed even the fallback split")


def _ensure_ntff_hook():
    """The agent image lacks antenv.axon_hooks; synthesize it (best effort) so
    a trace=True run (or a stray BASS_TRACE=1 env) never crashes on import."""
    import sys
    import types
    if "antenv.axon_hooks" in sys.modules:
        return
    try:
        from trn_agent_boot.trn_boot import _ntff_profile_via_ctypes
        hook = _ntff_profile_via_ctypes("/opt/axon/libaxon_pjrt.so")
    except Exception:
        hook = None
    try:
        mod = types.ModuleType("antenv.axon_hooks")
        mod.get_axon_ntff_profile_hook = lambda: hook
        mod.set_axon_ntff_profile_hook = lambda h: None
        sys.modules["antenv.axon_hooks"] = mod
        import antenv
        antenv.axon_hooks = mod
    except Exception:
        pass
    try:
        from concourse import bass_utils
        bass_utils.upload_artifacts = lambda tmpdir: tmpdir  # no S3 in sandbox
    except Exception:
        pass


def run_on_hw(feature_indices, feature_values, weight, bias, trace=False):
    from concourse import bass_utils
    _ensure_ntff_hook()
    nshards, u_pad, in_maps = _prep_in_maps(
        feature_indices, feature_values, weight, bias)
    nc = _get_nc(nshards, u_pad)
    res = bass_utils.run_bass_kernel_spmd(
        nc, in_maps, core_ids=list(range(NCORES)), trace=trace,
    )
    out = np.concatenate([r["out"] for r in res.results], axis=0)
    return out, res


def kernel(feature_indices, feature_values, weight, bias):
    out, _ = run_on_hw(feature_indices, feature_values, weight, bias,
                       trace=False)
    return out


# revision 19
# speedup vs baseline: 3.5380x; 1.1743x over previous
"""Trainium2 Bass kernel for FeatureTransformerSlice (embedding lookup).

out[b, :] = bias + sum_f mask(idx[b,f]) * val[b,f] * weight[max(idx[b,f],0), :]

Strategy (8 NeuronCores, data-parallel over batch):
  - Each core owns B/8 = 2048 batch rows, split into NSHARDS=2 shards of 1024
    rows.  Per shard the host remaps the used vocab ids (np.unique, ~22.6K <
    int16 max) to a compact bf16 table W[uniq] that lives in that core's HBM;
    bf16 halves the random-gather HBM traffic (1KB rows) and the 2e-2 rel-err
    budget dwarfs the 2^-9 rounding.
  - Gathers use the SWDGE dma_gather instruction: ONE GpSimd call fetches all
    4096 rows of a 128-row batch tile (32 features each) into a [128, 32, 512]
    bf16 SBUF tile (flat slot i = j*128+p lands at [p, j, :]).  16 calls/core
    replace the baseline's 512 indirect_dma_start calls, amortizing the ~1us
    SWDGE fixed overhead that serialized the whole kernel (GpSimd was 80% busy,
    DMA engines 50% idle waiting on descriptors).
  - Per tile the 32 features reduce on the PE as bf16 diag(val) matmuls
    accumulated in fp32 PSUM (bias enters via a K=1 ones x bias matmul);
    diagonals are built on DVE as val-broadcast * replicated-identity.  A few
    features can be peeled onto DVE (tensor_scalar + add) via DVE_FEATS to
    balance engines.  ACT copies PSUM to SBUF for the output DMA.
  - Indices arrive pre-wrapped from the host in the SWDGE layout (16-partition
    wrap replicated 8x across the 128 partitions); values arrive bf16 in
    feature-major [128, tiles*32] layout; masked (negative) features get val=0.
"""

import numpy as np
import ml_dtypes

bf16 = ml_dtypes.bfloat16

P = 128
B = 16384
F = 32
V = 40960
O = 512
NCORES = 8
BC = B // NCORES          # rows per core
TILES = BC // P           # batch tiles per core (16)
NI = P * F                # gathered rows per tile per call (4096)
S = NI // 16              # idx columns per tile in the 16-partition wrap (256)

# tuning knobs
NSHARDS = 2               # vocab-remap shards per core (2 -> ~22.6K uniq ids)
U_PAD = 23552             # padded compact-table rows (fits int16, > max uniq)
NSHARDS_FB = 4            # fallback if a shard overflows U_PAD
U_PAD_FB = 16384
DVE_FEATS = 8             # features per tile computed on DVE instead of PE
                          # (power of two; reduced via a binary tree of wide
                          # bf16 tensor_tensor ops, folded into PSUM via an
                          # identity matmul)
G_BUFS = 3                # gather-tile double/triple buffering
IDX_PER_CALL = 512        # rows per dma_gather call (divisor of NI, mult of 128;
                          # <= 1024 so one call's descriptors fit the 16KB
                          # SWDGE ring carveout; smaller = smoother DMA flow)
SWDGE_QUEUES = 4          # spread gather calls over SWDGE rings


def build_kernel(nshards=NSHARDS, u_pad=U_PAD, dve_feats=DVE_FEATS,
                 g_bufs=G_BUFS, idx_per_call=IDX_PER_CALL,
                 swdge_queues=SWDGE_QUEUES):
    import concourse.bacc as bacc
    import concourse.bass as bass
    import concourse.mybir as mybir
    import concourse.tile as tile

    f32 = mybir.dt.float32
    bf = mybir.dt.bfloat16
    i16 = mybir.dt.int16

    tiles_per_shard = TILES // nshards
    calls_per_tile = NI // idx_per_call
    j_per_call = idx_per_call // P

    nc = bacc.Bacc("TRN2", target_bir_lowering=False, debug=False,
                   num_swdge_queues=swdge_queues)

    w_ds = [nc.dram_tensor(f"w{h}", [u_pad, O], bf, kind="ExternalInput")
            for h in range(nshards)]
    ix_d = nc.dram_tensor("ix", [P, TILES * S], i16, kind="ExternalInput")
    vb_d = nc.dram_tensor("vb", [P, TILES * F], bf, kind="ExternalInput")
    b_d = nc.dram_tensor("b", [1, O], bf, kind="ExternalInput")
    id_d = nc.dram_tensor("id", [P, P], bf, kind="ExternalInput")
    out_d = nc.dram_tensor("out", [BC, O], f32, kind="ExternalOutput")

    with tile.TileContext(nc) as tc:
        with (
            tc.tile_pool(name="io", bufs=1) as io,
            tc.tile_pool(name="gp", bufs=g_bufs) as gp,
            tc.tile_pool(name="dp", bufs=2) as dp,
            tc.tile_pool(name="ob", bufs=3) as ob,
            tc.tile_pool(name="ps", bufs=4, space="PSUM") as ps,
        ):
            # ---- one-time loads ----
            ix_sb = io.tile([P, TILES * S], i16)
            nc.sync.dma_start(out=ix_sb[:], in_=ix_d.ap())
            vb_sb = io.tile([P, TILES * F], bf)
            nc.sync.dma_start(out=vb_sb[:], in_=vb_d.ap())
            bias_sb = io.tile([1, O], bf)
            nc.sync.dma_start(out=bias_sb[:], in_=b_d.ap())
            assert dve_feats == 0 or (dve_feats >= 2
                                      and dve_feats & (dve_feats - 1) == 0)
            ones_sb = io.tile([1, P], bf)
            nc.vector.memset(ones_sb[:], 1.0)
            pe_feats = F - dve_feats
            id_sb = io.tile([P, P], bf)
            nc.sync.dma_start(out=id_sb[:], in_=id_d.ap())

            # ---- main loop over batch tiles ----
            for t in range(TILES):
                w_d = w_ds[t // tiles_per_shard]
                G = gp.tile([P, F, O], bf, tag="g")
                for cc in range(calls_per_tile):
                    nc.gpsimd.dma_gather(
                        out_ap=G[:, cc * j_per_call:(cc + 1) * j_per_call, :],
                        in_ap=w_d.ap(),
                        idxs_ap=ix_sb[:, t * S + cc * (S // calls_per_tile):
                                      t * S + (cc + 1) * (S // calls_per_tile)],
                        num_idxs=idx_per_call,
                        num_idxs_reg=idx_per_call,
                        elem_size=O,
                        queue_num=(t * calls_per_tile + cc) % swdge_queues,
                    )

                d = dp.tile([P, pe_feats, P], bf, tag="d")
                vb_pe = vb_sb[:, t * F:t * F + pe_feats].unsqueeze(2)
                nc.vector.tensor_tensor(
                    out=d[:], in0=vb_pe.to_broadcast([P, pe_feats, P]),
                    in1=id_sb[:].unsqueeze(1).to_broadcast([P, pe_feats, P]),
                    op=mybir.AluOpType.mult,
                )

                acc = None
                if dve_feats:
                    sc = dp.tile([P, dve_feats, O], bf, tag="s")
                    vb_dv = vb_sb[:, t * F + pe_feats:t * F + F].unsqueeze(2)
                    nc.vector.tensor_tensor(
                        out=sc[:], in0=vb_dv.to_broadcast([P, dve_feats, O]),
                        in1=G[:, pe_feats:F, :], op=mybir.AluOpType.mult,
                    )
                    cur, width, lvl = sc, dve_feats, 0
                    while width > 1:
                        half = width // 2
                        nxt = dp.tile([P, half, O], bf, tag=f"t{lvl}")
                        nc.vector.tensor_tensor(
                            out=nxt[:], in0=cur[:, :half, :],
                            in1=cur[:, half:width, :], op=mybir.AluOpType.add,
                        )
                        cur, width, lvl = nxt, half, lvl + 1
                    acc = cur

                psum = ps.tile([P, O], f32)
                nc.tensor.matmul(
                    out=psum[:], lhsT=ones_sb[:], rhs=bias_sb[:],
                    start=True, stop=False,
                )
                for j in range(pe_feats):
                    nc.tensor.matmul(
                        out=psum[:], lhsT=d[:, j, :], rhs=G[:, j, :],
                        start=False, stop=acc is None and j == pe_feats - 1,
                    )
                if acc is not None:
                    nc.tensor.matmul(
                        out=psum[:], lhsT=id_sb[:], rhs=acc[:, 0, :],
                        start=False, stop=True,
                    )

                out_sb = ob.tile([P, O], f32, tag="o")
                nc.scalar.activation(
                    out=out_sb[:], in_=psum[:],
                    func=mybir.ActivationFunctionType.Copy,
                )
                nc.sync.dma_start(
                    out=out_d.ap()[t * P:(t + 1) * P, :], in_=out_sb[:],
                )

    nc.compile()
    return nc


_nc_cache = {}


def _get_nc(nshards, u_pad):
    key = (nshards, u_pad)
    if key not in _nc_cache:
        _nc_cache[key] = build_kernel(nshards=nshards, u_pad=u_pad)
    return _nc_cache[key]


def _pack_shard(idx_eff, val_eff, w_bf, u_pad):
    """Remap one shard's indices to a compact table.

    Returns (padded bf16 table [u_pad, O], wrapped int16 idx [tiles, P, S],
    bf16 val [P, tiles*F]) or None if the shard's vocab overflows u_pad."""
    rows = idx_eff.shape[0]
    tiles = rows // P
    uniq, inv = np.unique(idx_eff, return_inverse=True)
    if len(uniq) > min(u_pad, 32768):
        return None
    table = np.zeros((u_pad, O), bf16)
    table[:len(uniq)] = w_bf[uniq]
    idx16 = inv.reshape(rows, F).astype(np.int16)
    a = idx16.reshape(tiles, P, F).transpose(0, 2, 1)   # [t, j, p]: flat i=j*128+p
    a = a.reshape(tiles, NI).reshape(tiles, S, 16)      # [t, s, p16]: i=s*16+p16
    a = a.transpose(0, 2, 1)                            # [t, p16, s]
    ix = np.broadcast_to(a[:, None], (tiles, 8, 16, S)).reshape(tiles, P, S)
    v = val_eff.reshape(tiles, P, F).transpose(1, 0, 2).reshape(P, tiles * F)
    return table, np.ascontiguousarray(ix), v.astype(bf16)


def _prep_in_maps(feature_indices, feature_values, weight, bias):
    fi = np.asarray(feature_indices)
    fv = np.asarray(feature_values, dtype=np.float32)
    w_bf = np.asarray(weight, dtype=np.float32).astype(bf16)
    b_bf = np.asarray(bias, dtype=np.float32).astype(bf16).reshape(1, O)

    mask = fi >= 0
    val_eff = np.where(mask, fv, np.float32(0.0))
    idx_eff = np.where(mask, fi, 0).astype(np.int64)

    for nshards, u_pad in [(NSHARDS, U_PAD), (NSHARDS_FB, U_PAD_FB)]:
        ss = BC // nshards
        in_maps = []
        ok = True
        for c in range(NCORES):
            tables, ixs, vbs = [], [], []
            for h in range(nshards):
                lo = c * BC + h * ss
                packed = _pack_shard(idx_eff[lo:lo + ss], val_eff[lo:lo + ss],
                                     w_bf, u_pad)
                if packed is None:
                    ok = False
                    break
                tables.append(packed[0])
                ixs.append(packed[1])
                vbs.append(packed[2])
            if not ok:
                break
            m = {f"w{h}": tables[h] for h in range(nshards)}
            m["ix"] = np.concatenate(ixs, 0).transpose(1, 0, 2).reshape(
                P, TILES * S).copy()
            m["vb"] = np.concatenate(vbs, 1)
            m["b"] = b_bf
            m["id"] = np.eye(P, dtype=bf16)
            in_maps.append(m)
        if ok:
            return nshards, u_pad, in_maps
    raise RuntimeError("vocab shard overflowed even the fallback split")


def _ensure_ntff_hook():
    """The agent image lacks antenv.axon_hooks; synthesize it (best effort) so
    a trace=True run (or a stray BASS_TRACE=1 env) never crashes on import."""
    import sys
    import types
    if "antenv.axon_hooks" in sys.modules:
        return
    try:
        from trn_agent_boot.trn_boot import _ntff_profile_via_ctypes
        hook = _ntff_profile_via_ctypes("/opt/axon/libaxon_pjrt.so")
    except Exception:
        hook = None
    try:
        mod = types.ModuleType("antenv.axon_hooks")
        mod.get_axon_ntff_profile_hook = lambda: hook
        mod.set_axon_ntff_profile_hook = lambda h: None
        sys.modules["antenv.axon_hooks"] = mod
        import antenv
        antenv.axon_hooks = mod
    except Exception:
        pass
    try:
        from concourse import bass_utils
        bass_utils.upload_artifacts = lambda tmpdir: tmpdir  # no S3 in sandbox
    except Exception:
        pass


def run_on_hw(feature_indices, feature_values, weight, bias, trace=False):
    from concourse import bass_utils
    _ensure_ntff_hook()
    nshards, u_pad, in_maps = _prep_in_maps(
        feature_indices, feature_values, weight, bias)
    nc = _get_nc(nshards, u_pad)
    res = bass_utils.run_bass_kernel_spmd(
        nc, in_maps, core_ids=list(range(NCORES)), trace=trace,
    )
    out = np.concatenate([r["out"] for r in res.results], axis=0)
    return out, res


def kernel(feature_indices, feature_values, weight, bias):
    out, _ = run_on_hw(feature_indices, feature_values, weight, bias,
                       trace=False)
    return out
